# Initial kernel scaffold
#
"""Sliding-window multi-head attention (B=2, S=2048, E=512, H=8, Dh=64, W=64+1)
on 8 trn2 NeuronCores, sequence-parallel (256 queries/core + 32-halo), weights
replicated. Self-contained: takes full inputs, returns full output.

Per-core device program (all fp32r matmuls):
  qT/kT = Wq/Wk-stationary matmuls against host-pretransposed x^T   [E-major]
  V     = x^T-stationary matmul against Wv                           [S-major]
  scoresT[k,q] per (batch,head,k-chunk) -> exp(+keymask bias) on band rect
  -> band/query mask multiply -> attnV with ones-column (denominator row)
  -> reciprocal_approx + rank-1 PE broadcast -> normalize -> outproj (S-major)

PSUM: four [128,1024] (2-bank) rotating slots. Even heads' score tiles and odd
heads' score tiles live in different slots: concurrent PE row-group matmuls
writing one PSUM bank are a fatal HW race.
"""
import numpy as np

import concourse.bass as bass
import concourse.tile as tile
from concourse import bacc, mybir
from concourse.bass_utils import run_bass_kernel_spmd

F32 = mybir.dt.float32
F32R = mybir.dt.float32r

B, S, IN_DIM, EMBED, HEADS, WINDOW = 2, 2048, 512, 512, 8, 64
HALF = WINDOW // 2          # 32
N_CORES = 8
QC = S // N_CORES           # 256 queries per core
KW = QC + 2 * HALF          # 320 key rows per core window
EPS = 1e-9

# per k-chunk band rectangles: (kw, rect_q0, rect_w, band_col_off)
CHUNKS = [(128, 0, 128, 0), (128, 64, 192, 128), (64, 192, 64, 320)]
BAND_W = 384


def build_nc(reps: int = 1, skip_norm: bool = False, skip_expmul: bool = False):
    nc = bacc.Bacc("TRN2", target_bir_lowering=False, debug=False, num_devices=N_CORES)

    xT = nc.dram_tensor("xT", [B, IN_DIM, KW], F32R, kind="ExternalInput").ap()
    Wq = nc.dram_tensor("Wq", [IN_DIM, EMBED], F32R, kind="ExternalInput").ap()
    Wk = nc.dram_tensor("Wk", [IN_DIM, EMBED], F32R, kind="ExternalInput").ap()
    Wv = nc.dram_tensor("Wv", [IN_DIM, EMBED], F32R, kind="ExternalInput").ap()
    Wo = nc.dram_tensor("Wo", [EMBED, EMBED], F32R, kind="ExternalInput").ap()
    bq = nc.dram_tensor("bq", [EMBED], F32, kind="ExternalInput").ap()
    bk = nc.dram_tensor("bk", [EMBED], F32, kind="ExternalInput").ap()
    bv = nc.dram_tensor("bv", [1, EMBED], F32R, kind="ExternalInput").ap()
    bo = nc.dram_tensor("bo", [1, EMBED], F32R, kind="ExternalInput").ap()
    km = nc.dram_tensor("km", [B, 384], F32, kind="ExternalInput").ap()
    band = nc.dram_tensor("band", [B, 128, BAND_W], F32, kind="ExternalInput").ap()
    out = nc.dram_tensor("out", [B, QC, EMBED], F32, kind="ExternalOutput").ap()

    with tile.TileContext(nc) as tc:
        with tc.tile_pool(name="const", bufs=1) as cpool, \
             tc.tile_pool(name="work", bufs=1) as wpool, \
             tc.tile_pool(name="psum", bufs=4, space="PSUM") as ppool:

            def pt(name):
                return ppool.tile([128, 1024], F32, tag="big", name=name)

            # ---- constant loads -----------------------------------------------
            W_sb = {}
            for name, src in (("q", Wq), ("k", Wk), ("v", Wv), ("o", Wo)):
                for kc in range(4):
                    t = cpool.tile([128, EMBED], F32R, tag=f"W{name}{kc}", name=f"W{name}{kc}")
                    nc.sync.dma_start(t[:], src[128 * kc:128 * (kc + 1), :])
                    W_sb[name, kc] = t
            xT_sb = {}
            for b in range(B):
                for kc in range(4):
                    t = cpool.tile([128, KW], F32R, tag=f"xT{b}{kc}", name=f"xT{b}{kc}")
                    nc.sync.dma_start(t[:], xT[b, 128 * kc:128 * (kc + 1), :])
                    xT_sb[b, kc] = t
            bq_sb = cpool.tile([128, 4], F32, tag="bq")
            nc.sync.dma_start(bq_sb[:], bq.rearrange("(m p) -> p m", p=128))
            bk_sb = cpool.tile([128, 4], F32, tag="bk")
            nc.sync.dma_start(bk_sb[:], bk.rearrange("(m p) -> p m", p=128))
            bv_sb = cpool.tile([1, EMBED], F32R, tag="bv")
            nc.sync.dma_start(bv_sb[:], bv[:])
            bo_sb = cpool.tile([1, EMBED], F32R, tag="bo")
            nc.sync.dma_start(bo_sb[:], bo[:])
            km_sb = {}
            for b in range(B):
                t = cpool.tile([128, 3], F32, tag=f"km{b}", name=f"km{b}")
                nc.sync.dma_start(t[:], km[b].rearrange("(c p) -> p c", p=128))
                km_sb[b] = t
            band_sb = {}
            for b in range(B):
                t = cpool.tile([128, BAND_W], F32, tag=f"band{b}", name=f"band{b}")
                nc.sync.dma_start(t[:], band[b])
                band_sb[b] = t

            # f32 scratch constants (memset can not write fp32r; tensor_copy rounds)
            zscr = wpool.tile([128, 2048], F32, tag="zscr")
            nc.vector.memset(zscr[:], 0.0)
            onescr = wpool.tile([128, 128], F32, tag="onescr")
            nc.vector.memset(onescr[:], 1.0)
            epsscr = wpool.tile([1, 2048], F32, tag="epsscr")
            nc.vector.memset(epsscr[:], EPS)

            ones_row = cpool.tile([1, 128], F32R, tag="ones")
            nc.vector.tensor_copy(ones_row[:], onescr[0:1, :])

            # attnT tiles [128, 8*QC]; col block j hosts head blk(h)=(h%2)*4+h//2.
            # zero once; chunk-2 epsilon row feeds the denominator.
            attnT = []
            for c in range(3):
                t = cpool.tile([128, 8 * QC], F32R, tag=f"attnT{c}", name=f"attnT{c}")
                nc.vector.tensor_copy(t[:], zscr[:])
                attnT.append(t)
            nc.vector.tensor_copy(attnT[2][64:65, :], epsscr[:])

            # V tiles with interleaved ones-columns [128, 8*65]
            V_sb = {}
            for b in range(B):
                for sc in range(3):
                    t = cpool.tile([128, 8 * 65], F32R, tag=f"V{b}{sc}", name=f"V{b}{sc}")
                    nc.vector.tensor_copy(t[:], zscr[:, 0:520])
                    nc.vector.tensor_copy(
                        t[:].rearrange("p (h c) -> p h c", h=8)[:, :, 64:65],
                        onescr[:, 0:1].rearrange("p (o w) -> p o w", o=1).broadcast_to([128, 8, 1]))
                    V_sb[b, sc] = t

            qT_sb = [cpool.tile([128, 4 * QC], F32R, tag=f"qT{b}", name=f"qT{b}") for b in range(B)]
            kT_sb = [cpool.tile([128, 4 * KW], F32R, tag=f"kT{b}", name=f"kT{b}") for b in range(B)]
            ao_sb = [cpool.tile([128, 4 * QC], F32R, tag=f"ao{b}", name=f"ao{b}") for b in range(B)]

            # ---- main body (repeatable for timing) ----------------------------
            for rep in range(reps):
                for b in range(B):
                    # ---- QKV projections ----
                    tQ = pt("tQ")
                    for m in range(4):
                        ps = tQ[:, m * 256:m * 256 + QC]
                        for kc in range(4):
                            nc.tensor.matmul(ps, W_sb["q", kc][:, 128 * m:128 * (m + 1)],
                                             xT_sb[b, kc][:, HALF:HALF + QC],
                                             start=(kc == 0), stop=(kc == 3))
                        nc.scalar.add(qT_sb[b][:, m * QC:(m + 1) * QC], ps, bq_sb[:, m:m + 1])
                    tKs = [pt("tK0"), pt("tK1")]
                    for m in range(4):
                        ps = tKs[m // 2][:, (m % 2) * 512:(m % 2) * 512 + KW]
                        for kc in range(4):
                            nc.tensor.matmul(ps, W_sb["k", kc][:, 128 * m:128 * (m + 1)],
                                             xT_sb[b, kc][:, 0:KW],
                                             start=(kc == 0), stop=(kc == 3))
                        nc.scalar.add(kT_sb[b][:, m * KW:(m + 1) * KW], ps, bk_sb[:, m:m + 1])
                    tVs = [pt("tV0"), pt("tV1")]
                    for sc in range(3):
                        sw = min(128, KW - 128 * sc)
                        ps = tVs[sc // 2][0:sw, (sc % 2) * 512:(sc % 2) * 512 + 512]
                        for kc in range(4):
                            nc.tensor.matmul(ps, xT_sb[b, kc][:, 128 * sc:128 * sc + sw],
                                             W_sb["v", kc][:], start=(kc == 0), stop=False)
                        nc.tensor.matmul(ps, ones_row[0:1, 0:sw], bv_sb[:], start=False, stop=True)
                        nc.vector.tensor_copy(
                            V_sb[b, sc][0:sw].rearrange("p (h c) -> p h c", h=8)[:, :, 0:64],
                            ps.rearrange("p (h d) -> p h d", h=8))

                    # ---- banded attention ----
                    # outT accumulators: tOs[0] hosts heads 0-3 (col h*256),
                    # tOs[1] heads 4-7; denominator lands on partition row 64.
                    tOs = [pt("tO0"), pt("tO1")]
                    for ci, (kw, rq0, rw, boff) in enumerate(CHUNKS):
                        # scores: even heads -> tSe, odd heads -> tSo (bank-disjoint
                        # slots; concurrent row-group MMs must not share a bank)
                        tSe, tSo = pt("tSe"), pt("tSo")
                        for h in range(8):
                            bp = (h % 2) * 64
                            tS = tSe if h % 2 == 0 else tSo
                            nc.tensor.matmul(
                                tS[0:kw, (h // 2) * QC:(h // 2 + 1) * QC],
                                kT_sb[b][bp:bp + 64, (h // 2) * KW + 128 * ci:(h // 2) * KW + 128 * ci + kw],
                                qT_sb[b][bp:bp + 64, (h // 2) * QC:(h // 2 + 1) * QC],
                                start=True, stop=True)
                        for half, tS in ((0, tSe), (1, tSo)):
                            in3 = tS[0:kw, :].rearrange("p (h q) -> p h q", h=4)[:, :, rq0:rq0 + rw]
                            out3 = attnT[ci][0:kw, half * 1024:(half + 1) * 1024].rearrange(
                                "p (h q) -> p h q", h=4)[:, :, rq0:rq0 + rw]
                            if skip_expmul:
                                nc.scalar.activation(out3, in3, mybir.ActivationFunctionType.Copy)
                            else:
                                nc.scalar.activation(out3, in3, mybir.ActivationFunctionType.Exp,
                                                     bias=km_sb[b][0:kw, ci:ci + 1])
                                b3 = band_sb[b][0:kw, boff:boff + rw].rearrange(
                                    "p (o w) -> p o w", o=1).broadcast_to([kw, 4, rw])
                                nc.vector.tensor_tensor(out3, out3, b3, mybir.AluOpType.mult)
                    # attnV: one open accumulation group per PSUM bank -> parity passes
                    for parity in range(2):
                        for ci, (kw, _, _, _) in enumerate(CHUNKS):
                            kwa = kw + 1 if ci == 2 else kw
                            for h in range(parity, 8, 2):
                                sb = ((h % 2) * 4 + h // 2) * QC
                                nc.tensor.matmul(
                                    tOs[h // 4][0:65, (h % 4) * QC:(h % 4 + 1) * QC],
                                    V_sb[b, ci][0:kwa, h * 65:(h + 1) * 65],
                                    attnT[ci][0:kwa, sb:sb + QC],
                                    start=(ci == 0), stop=(ci == 2))

                    # ---- normalize ----
                    if skip_norm:
                        for h in range(8):
                            nc.vector.tensor_copy(
                                ao_sb[b][(h % 2) * 64:(h % 2) * 64 + 64, (h // 2) * QC:(h // 2 + 1) * QC],
                                tOs[h // 4][0:64, (h % 4) * QC:(h % 4 + 1) * QC])
                    else:
                        drow = wpool.tile([1, 2048], F32, tag="drow")
                        nc.scalar.activation(drow[0:1, 0:1024], tOs[0][64:65, :],
                                             mybir.ActivationFunctionType.Copy)
                        nc.scalar.activation(drow[0:1, 1024:2048], tOs[1][64:65, :],
                                             mybir.ActivationFunctionType.Copy)
                        rrow = wpool.tile([1, 2048], F32, tag="rrow")
                        nc.vector.reciprocal_approx_fast(rrow[:], drow[:])
                        rrow_r = wpool.tile([1, 2048], F32R, tag="rrow_r")
                        nc.vector.tensor_copy(rrow_r[:], rrow[:])
                        tRs = [pt("tR0"), pt("tR1")]
                        rb = wpool.tile([64, 2048], F32, tag="rb")
                        for g in range(2):
                            for j in range(4):
                                nc.tensor.matmul(tRs[g][0:64, j * QC:(j + 1) * QC],
                                                 ones_row[0:1, 0:64],
                                                 rrow_r[0:1, g * 1024 + j * QC:g * 1024 + (j + 1) * QC],
                                                 start=True, stop=True)
                            nc.scalar.activation(rb[0:64, g * 1024:(g + 1) * 1024], tRs[g][0:64, :],
                                                 mybir.ActivationFunctionType.Copy)
                        for h in range(8):
                            nc.vector.tensor_tensor(
                                ao_sb[b][(h % 2) * 64:(h % 2) * 64 + 64, (h // 2) * QC:(h // 2 + 1) * QC],
                                tOs[h // 4][0:64, (h % 4) * QC:(h % 4 + 1) * QC],
                                rb[0:64, h * QC:(h + 1) * QC],
                                mybir.AluOpType.mult)

                    # ---- output projection (S-major out) ----
                    tP = pt("tP")
                    for qc in range(2):
                        ps = tP[:, qc * 512:(qc + 1) * 512]
                        for ec in range(4):
                            nc.tensor.matmul(ps, ao_sb[b][:, ec * QC + 128 * qc:ec * QC + 128 * qc + 128],
                                             W_sb["o", ec][:], start=(ec == 0), stop=False)
                        nc.tensor.matmul(ps, ones_row[0:1, 0:128], bo_sb[:], start=False, stop=True)
                        osb = wpool.tile([128, EMBED], F32, tag=f"osb{qc}", name=f"osb{qc}")
                        nc.vector.tensor_copy(osb[:], ps)
                        nc.sync.dma_start(out[b, 128 * qc:128 * (qc + 1), :], osb[:])

    nc.compile()
    return nc


def host_prep(x, padding_mask, Wqkv, bqkv, Wo, bo):
    """Build per-core input maps (numpy only)."""
    x = np.asarray(x, dtype=np.float32)
    pm = np.asarray(padding_mask) != 0
    Wqkv = np.asarray(Wqkv, dtype=np.float32)
    bqkv = np.asarray(bqkv, dtype=np.float32)
    Wo_np = np.asarray(Wo, dtype=np.float32)
    bo_np = np.asarray(bo, dtype=np.float32)

    hidx = np.arange(HEADS).repeat(64) * 192 + np.tile(np.arange(64), HEADS)
    Wq = np.ascontiguousarray(Wqkv[:, hidx]) / 8.0
    Wk = np.ascontiguousarray(Wqkv[:, hidx + 64])
    Wv = np.ascontiguousarray(Wqkv[:, hidx + 128])
    bq = np.ascontiguousarray(bqkv[hidx]) / 8.0
    bk = np.ascontiguousarray(bqkv[hidx + 64])
    bv = np.ascontiguousarray(bqkv[hidx + 128])[None, :]
    bo2 = bo_np[None, :]

    x_pad = np.zeros((B, S + 2 * HALF, IN_DIM), np.float32)
    x_pad[:, HALF:HALF + S] = x
    pm_pad = np.zeros((B, S + 2 * HALF), bool)
    pm_pad[:, HALF:HALF + S] = pm

    in_maps = []
    for c in range(N_CORES):
        q0 = QC * c
        xT_c = np.ascontiguousarray(x_pad[:, q0:q0 + KW, :].transpose(0, 2, 1))
        km_c = np.zeros((B, 384), np.float32)
        km_c[:, :KW] = np.where(pm_pad[:, q0:q0 + KW], 0.0, -1e9)
        km_c[:, KW:] = -1e9
        band_c = np.zeros((B, 128, BAND_W), np.float32)
        for ci, (kw, rq0, rw, boff) in enumerate(CHUNKS):
            kk = np.arange(128)[:, None]
            jj = np.arange(rw)[None, :]
            krel = 128 * ci + kk
            qq = rq0 + jj
            geo = (krel - qq >= 0) & (krel - qq <= 64) & (kk < kw)
            qpad = pm[:, q0 + rq0:q0 + rq0 + rw]  # [B, rw]
            band_c[:, :, boff:boff + rw] = geo[None] * qpad[:, None, :]
        in_maps.append({
            "xT": xT_c, "Wq": Wq, "Wk": Wk, "Wv": Wv, "Wo": Wo_np,
            "bq": bq, "bk": bk, "bv": bv, "bo": bo2,
            "km": km_c, "band": band_c,
        })
    return in_maps


_NC_CACHE = {}


def kernel(x, padding_mask, Wqkv, bqkv, Wo, bo):
    if "nc" not in _NC_CACHE:
        _NC_CACHE["nc"] = build_nc(reps=1)
    nc = _NC_CACHE["nc"]
    in_maps = host_prep(x, padding_mask, Wqkv, bqkv, Wo, bo)
    res = run_bass_kernel_spmd(nc, in_maps, core_ids=list(range(N_CORES)), trace=False)
    full = np.empty((B, S, EMBED), np.float32)
    for c in range(N_CORES):
        full[:, QC * c:QC * (c + 1), :] = res.results[c]["out"]
    return full



# revision 4
# speedup vs baseline: 2.9810x; 2.9810x over previous
"""Sliding-window MHA (B=2, S=2048, E=512, H=8, Dh=64, W=64+1) on 8 trn2
NeuronCores, sequence-parallel (256 queries/core + 32-halo), weights replicated.

v2: fp16 matmul path (PE: 1 cyc/row at any free size), rect-tight score and
attnV matmuls (6-split banded accumulation), softmax normalization via
sbuf-row reciprocal_approx_fast + fp16 rank-1 PE broadcast + single-PSUM
multiply, PSUM->SBUF moves spread across ACT/DVE (GPSIMD cannot touch PSUM),
batches interleaved in emission order so PE stays busy during ACT/DVE phases.
All matmul PSUM outputs are partition-0, single-bank, non-crossing (HW req).

Layouts per core:
  qT  [128, 4*256] fp16: partition=(h%2)*64+d, col block m=h//2 (E-major)
  kT  [128, 4*320] fp16: same, KW=320 keys
  V   [sw, 8*65]   fp16 per 128-key chunk: col h*65+d, col h*65+64 = ones
  attnT[ci] [128, 8*rw] fp16: col g*rw+q-rect, g=(h%2)*4+h//2
  scores psum per chunk: even heads g0..3 / odd g4..7 in bank-disjoint ranges
  tO [65, 4*256] f32 psum (e/o): row 64 = softmax denominator
  ao [128, 4*256] fp16: E-major normalized attention output
  out-proj: S-major psum [128 q, 512] -> sbuf f32 -> DMA
"""
import numpy as np

import concourse.bass as bass
import concourse.tile as tile
from concourse import bacc, mybir
from concourse.bass_utils import run_bass_kernel_spmd

F32 = mybir.dt.float32
F32R = mybir.dt.float32r
F16 = mybir.dt.float16

B, S, IN_DIM, EMBED, HEADS, WINDOW = 2, 2048, 512, 512, 8, 64
HALF = WINDOW // 2          # 32
N_CORES = 8
QC = S // N_CORES           # 256 queries per core
KW = QC + 2 * HALF          # 320 key rows per core window
EPS = 2e-5

# chunks along the key axis: (kw, rect_q0, rect_w, band_col_off)
CHUNKS = [(128, 0, 128, 0), (128, 64, 192, 128), (64, 192, 64, 320)]
BAND_W = 384
# attnV accumulation regions: (q0, ci, rect_off, start, stop); width 64 each
AV_REGIONS = [
    (0, 0, 0, True, True),
    (64, 0, 64, True, False), (64, 1, 0, False, True),
    (128, 1, 64, True, True),
    (192, 1, 128, True, False), (192, 2, 0, False, True),
]
# av_mode="wide": attnT[1] zero-padded to [128, 8*256]; 3 matmuls per head
# (c1 full-width first with start, c0/c2 tight adds)
AV_WIDE = [
    (1, 0, 256, True, False),   # (ci, q0, width, start, stop); rect_off = q0-rq0
    (0, 0, 128, False, False),
    (2, 192, 64, False, True),
]


def build_nc(reps: int = 1, with_bias: bool = False, av_mode: str = "split"):
    nc = bacc.Bacc("TRN2", target_bir_lowering=False, debug=False, num_devices=N_CORES)

    # packed layouts: contraction-chunk kc along the free dim (one DMA each)
    xT = nc.dram_tensor("xT", [B, 128, 4 * KW], F16, kind="ExternalInput").ap()
    Wq = nc.dram_tensor("Wq", [128, 4 * EMBED], F16, kind="ExternalInput").ap()
    Wk = nc.dram_tensor("Wk", [128, 4 * EMBED], F16, kind="ExternalInput").ap()
    Wv = nc.dram_tensor("Wv", [128, 4 * EMBED], F16, kind="ExternalInput").ap()
    Wo = nc.dram_tensor("Wo", [128, 4 * EMBED], F16, kind="ExternalInput").ap()
    km = nc.dram_tensor("km", [B, 384], F32, kind="ExternalInput").ap()
    band = nc.dram_tensor("band", [B, 128, BAND_W], F16, kind="ExternalInput").ap()
    out = nc.dram_tensor("out", [B, QC, EMBED], F32, kind="ExternalOutput").ap()
    if with_bias:
        bq = nc.dram_tensor("bq", [1, EMBED], F16, kind="ExternalInput").ap()
        bk = nc.dram_tensor("bk", [1, EMBED], F16, kind="ExternalInput").ap()
        bv = nc.dram_tensor("bv", [1, EMBED], F16, kind="ExternalInput").ap()
        bo = nc.dram_tensor("bo", [1, EMBED], F16, kind="ExternalInput").ap()

    with tile.TileContext(nc) as tc:
        with tc.tile_pool(name="const", bufs=1) as cpool, \
             tc.tile_pool(name="work", bufs=1) as wpool, \
             tc.tile_pool(name="psum", bufs=4, space="PSUM") as ppool:

            def pt(name):
                return ppool.tile([128, 1024], F32, tag="big", name=name)

            # ---- constant loads (consumption order, packed) -------------------
            W_sb, xT_sb, km_sb, band_sb = {}, {}, {}, {}

            def load_w(name, src):
                t = cpool.tile([128, 4 * EMBED], F16, tag=f"W{name}", name=f"W{name}")
                nc.sync.dma_start(t[:], src[:])
                W_sb[name] = t

            def wsl(name, kc, c0, w):
                return W_sb[name][:, kc * EMBED + c0:kc * EMBED + c0 + w]

            def load_x(b):
                # second HWDGE queue (Activation) runs in parallel with SP
                t = cpool.tile([128, 4 * KW], F16, tag=f"xT{b}", name=f"xT{b}")
                nc.scalar.dma_start(t[:], xT[b])
                xT_sb[b] = t

            def xsl(b, kc, c0, w):
                return xT_sb[b][:, kc * KW + c0:kc * KW + c0 + w]

            def load_masks(b):
                t = cpool.tile([128, 3], F32, tag=f"km{b}", name=f"km{b}")
                nc.scalar.dma_start(t[:], km[b].rearrange("(c p) -> p c", p=128))
                km_sb[b] = t
                t = cpool.tile([128, BAND_W], F16, tag=f"band{b}", name=f"band{b}")
                nc.scalar.dma_start(t[:], band[b])
                band_sb[b] = t

            load_w("q", Wq)
            load_x(0)
            load_w("k", Wk)
            load_w("v", Wv)
            load_masks(0)
            load_w("o", Wo)
            load_x(1)
            load_masks(1)
            if with_bias:
                b_sb = {}
                for name, src in (("q", bq), ("k", bk), ("v", bv), ("o", bo)):
                    t = cpool.tile([1, EMBED], F16, tag=f"b{name}", name=f"b{name}")
                    nc.sync.dma_start(t[:], src[:])
                    b_sb[name] = t

            # scratch constants
            onescr = wpool.tile([128, 64], F32, tag="onescr")
            nc.vector.memset(onescr[:], 1.0)
            ones64_16 = cpool.tile([1, 64], F16, tag="ones64h")
            nc.vector.tensor_copy(ones64_16[:], onescr[0:1, :])
            if with_bias:
                ones16 = cpool.tile([1, QC], F16, tag="ones16")
                nc.vector.memset(ones16[:], 1.0)

            # V tiles with ones-columns at h*65+64 (set once)
            V_sb = {}
            for b in range(B):
                for sc in range(3):
                    t = cpool.tile([128, 8 * 65], F16, tag=f"V{b}{sc}", name=f"V{b}{sc}")
                    nc.vector.tensor_copy(
                        t[:].rearrange("p (h c) -> p h c", h=8)[:, :, 64:65],
                        onescr[:, 0:1].rearrange("p (o w) -> p o w", o=1).broadcast_to([128, 8, 1]))
                    V_sb[b, sc] = t

            # attnT per chunk fp16 (rects fully rewritten each rep). In "wide"
            # av_mode chunk 1 is zero-padded to full 256-wide head blocks so a
            # single full-width matmul with start=True opens each head's
            # accumulation; the pad region is never written after setup.
            zscr = wpool.tile([128, 2048], F32, tag="zscr")
            nc.vector.memset(zscr[:], 0.0)
            attnT = {}
            AT_W = {}   # head-block width per chunk
            AT_OFF = {}  # col offset of rect within block
            for ci, (kw, rq0, rw, boff) in enumerate(CHUNKS):
                for b in range(B):
                    if av_mode == "wide" and ci == 1:
                        AT_W[ci], AT_OFF[ci] = 256, 64
                        t = cpool.tile([128, 8 * 256], F16, tag=f"attnT{b}{ci}", name=f"attnT{b}{ci}")
                        nc.vector.tensor_copy(t[:], zscr[:])
                    else:
                        AT_W[ci], AT_OFF[ci] = rw, 0
                        t = cpool.tile([128, 8 * rw], F16, tag=f"attnT{b}{ci}", name=f"attnT{b}{ci}")
                    attnT[b, ci] = t

            qT_sb = [cpool.tile([128, 4 * QC], F16, tag=f"qT{b}", name=f"qT{b}") for b in range(B)]
            kT_sb = [cpool.tile([128, 4 * KW], F16, tag=f"kT{b}", name=f"kT{b}") for b in range(B)]
            # unnormalized attention output staging (psum->sbuf: tensor ops
            # may read at most one PSUM operand, and this frees tO slots early)
            aoun_sb = [cpool.tile([128, 4 * QC], F32, tag=f"aoun{b}", name=f"aoun{b}")
                       for b in range(B)]
            ao_sb = [cpool.tile([128, 4 * QC], F16, tag=f"ao{b}", name=f"ao{b}") for b in range(B)]
            drow_sb = [cpool.tile([1, 2048], F32, tag=f"drow{b}", name=f"drow{b}") for b in range(B)]
            rrow_sb = [cpool.tile([1, 2048], F32, tag=f"rrow{b}", name=f"rrow{b}") for b in range(B)]
            rrow16_sb = [cpool.tile([1, 2048], F16, tag=f"rrow16{b}", name=f"rrow16{b}") for b in range(B)]
            osb = [cpool.tile([128, 2 * EMBED], F32, tag=f"osb{b}", name=f"osb{b}")
                   for b in range(B)]

            # score psum column offsets per chunk for even/odd head groups
            SCOL = {0: (0, 512), 1: (0, 0), 2: (0, 512)}  # c1 uses two tiles

            # ---------- emission helpers ----------
            def emit_qproj(b):
                tQ = pt(f"tQ{b}")
                for m in range(4):
                    ps = tQ[:, m * 256:m * 256 + QC]
                    for kc in range(4):
                        nc.tensor.matmul(ps, wsl("q", kc, 128 * m, 128),
                                         xsl(b, kc, HALF, QC),
                                         start=(kc == 0), stop=(kc == 3 and not with_bias))
                    if with_bias:
                        nc.tensor.matmul(ps, b_sb["q"][0:1, 128 * m:128 * m + 128],
                                         ones16[0:1, :], start=False, stop=True)
                nc.scalar.copy(qT_sb[b][:], tQ[:])

            def emit_kproj(b):
                for half in range(2):
                    tK = pt(f"tK{b}{half}")
                    for mi in range(2):
                        m = half * 2 + mi
                        ps = tK[:, mi * 512:mi * 512 + KW]  # bank-aligned
                        for kc in range(4):
                            nc.tensor.matmul(ps, wsl("k", kc, 128 * m, 128),
                                             xsl(b, kc, 0, KW),
                                             start=(kc == 0), stop=(kc == 3 and not with_bias))
                        if with_bias:
                            nc.tensor.matmul(ps, b_sb["k"][0:1, 128 * m:128 * m + 128],
                                             ones16[0:1, 0:KW], start=False, stop=True)
                    nc.vector.tensor_copy(
                        kT_sb[b][:, half * 2 * KW:(half + 1) * 2 * KW].rearrange(
                            "p (c k) -> p c k", c=2),
                        tK[:].rearrange("p (c k) -> p c k", c=2)[:, :, 0:KW])

            def emit_vproj(b, scs):
                # V projection for the listed 128-key chunks (one psum slot)
                tV = pt(f"tV{b}{scs[0]}")
                for i, sc in enumerate(scs):
                    sw = min(128, KW - 128 * sc)
                    ps = tV[0:sw, i * 512:i * 512 + 512]
                    for kc in range(4):
                        nc.tensor.matmul(ps, xsl(b, kc, 128 * sc, sw),
                                         wsl("v", kc, 0, 512), start=(kc == 0),
                                         stop=(kc == 3 and not with_bias))
                    if with_bias:
                        nc.tensor.matmul(ps, ones16[0:1, 0:sw], b_sb["v"][:],
                                         start=False, stop=True)
                    nc.vector.tensor_copy(
                        V_sb[b, sc][0:sw].rearrange("p (h c) -> p h c", h=8)[:, :, 0:64],
                        ps.rearrange("p (h d) -> p h d", h=8))

            def emit_exp_mask(b, ci, tS, soff, kw, rw, boff, nh, gofs, stride=None):
                # exp(+key-mask bias) then band multiply for nh head blocks;
                # soff = offset of the first block WITHIN a stride-sized slot
                st = stride if stride is not None else rw
                in3 = tS[0:kw, 0:nh * st].rearrange(
                    "p (h q) -> p h q", h=nh)[:, :, soff:soff + rw] if st != rw else \
                    tS[0:kw, soff:soff + nh * rw].rearrange("p (h q) -> p h q", h=nh)
                aw, aoff = AT_W[ci], AT_OFF[ci]
                out3 = attnT[b, ci][0:kw, gofs * aw:(gofs + nh) * aw].rearrange(
                    "p (h q) -> p h q", h=nh)[:, :, aoff:aoff + rw]
                nc.scalar.activation(out3, in3, mybir.ActivationFunctionType.Exp,
                                     bias=km_sb[b][0:kw, ci:ci + 1])
                b3 = band_sb[b][0:kw, boff:boff + rw].rearrange(
                    "p (o w) -> p o w", o=1).broadcast_to([kw, nh, rw])
                nc.vector.tensor_tensor(out3, out3, b3, mybir.AluOpType.mult)

            def emit_score_mms(b, ci, tiles, offs, stride=None):
                kw, rq0, rw, _ = CHUNKS[ci]
                st = stride if stride is not None else rw
                for h in range(8):
                    bp = (h % 2) * 64
                    tS, off = tiles[h % 2], offs[h % 2]
                    nc.tensor.matmul(
                        tS[0:kw, off + (h // 2) * st: off + (h // 2) * st + rw],
                        kT_sb[b][bp:bp + 64, (h // 2) * KW + 128 * ci:(h // 2) * KW + 128 * ci + kw],
                        qT_sb[b][bp:bp + 64, (h // 2) * QC + rq0:(h // 2) * QC + rq0 + rw],
                        start=True, stop=True)

            def emit_scores(b):
                # c0: one tile, even [0:512] / odd [512:1024] (bank-disjoint),
                # merged exp+mask across all 8 blocks.
                tS0 = pt(f"tS{b}0")
                emit_score_mms(b, 0, (tS0, tS0), (0, 512))
                emit_exp_mask(b, 0, tS0, 0, 128, 128, 0, 8, 0)
                # c1: two tiles, blocks at 256-stride (width 192, no bank
                # crossing); c2 rides in the 64-wide gaps at +192.
                tS1e, tS1o = pt(f"tS{b}1e"), pt(f"tS{b}1o")
                emit_score_mms(b, 1, (tS1e, tS1o), (0, 0), stride=256)
                emit_exp_mask(b, 1, tS1e, 0, 128, 192, 128, 4, 0, stride=256)
                emit_exp_mask(b, 1, tS1o, 0, 128, 192, 128, 4, 4, stride=256)
                emit_score_mms(b, 2, (tS1e, tS1o), (192, 192), stride=256)
                emit_exp_mask(b, 2, tS1e, 192, 64, 64, 320, 4, 0, stride=256)
                emit_exp_mask(b, 2, tS1o, 192, 64, 64, 320, 4, 4, stride=256)

            def emit_attnv(b, tOs=None):
                if tOs is None:
                    tOs = (pt(f"tOe{b}"), pt(f"tOo{b}"))
                # evens first: the odd tile's slot frees later (WAR on b1 exps)
                # and drow-e fires as soon as the even half is done
                for h in (0, 2, 4, 6, 1, 3, 5, 7):
                    g = (h % 2) * 4 + h // 2
                    tO = tOs[h % 2]
                    if av_mode == "split":
                        for (q0, ci, roff, s, e) in AV_REGIONS:
                            kw_c = CHUNKS[ci][0]
                            nc.tensor.matmul(
                                tO[0:65, (h // 2) * QC + q0:(h // 2) * QC + q0 + 64],
                                V_sb[b, ci][0:kw_c, h * 65:(h + 1) * 65],
                                attnT[b, ci][0:kw_c, g * AT_W[ci] + roff:g * AT_W[ci] + roff + 64],
                                start=s, stop=e)
                    else:
                        for (ci, q0, w, s, e) in AV_WIDE:
                            kw_c = CHUNKS[ci][0]
                            bc = AT_OFF[ci] + q0 - CHUNKS[ci][1]
                            nc.tensor.matmul(
                                tO[0:65, (h // 2) * QC + q0:(h // 2) * QC + q0 + w],
                                V_sb[b, ci][0:kw_c, h * 65:(h + 1) * 65],
                                attnT[b, ci][0:kw_c, g * AT_W[ci] + bc:g * AT_W[ci] + bc + w],
                                start=s, stop=e, skip_group_check=True)
                return tOs

            def emit_drow(b, tOs):
                # denominator rows -> sbuf f32r (+eps) on ACT; unnormalized
                # attention output -> sbuf (ACT even / DVE odd), freeing the
                # tO psum slots early
                nc.scalar.activation(drow_sb[b][0:1, 0:1024], tOs[0][64:65, :],
                                     mybir.ActivationFunctionType.Copy, bias=EPS)
                nc.scalar.copy(aoun_sb[b][0:64, :], tOs[0][0:64, :])
                nc.scalar.activation(drow_sb[b][0:1, 1024:2048], tOs[1][64:65, :],
                                     mybir.ActivationFunctionType.Copy, bias=EPS)
                nc.vector.tensor_copy(aoun_sb[b][64:128, :], tOs[1][0:64, :])

            def emit_norm(b, tOs):
                # rrow = 1/denom on the SBUF row (proven op placement), then
                # PE-broadcast reciprocals to psum, then ao = aoun * rdb
                # (one PSUM operand per tensor_tensor)
                for parity in range(2):
                    nc.vector.reciprocal_approx_fast(
                        rrow_sb[b][0:1, parity * 1024:(parity + 1) * 1024],
                        drow_sb[b][0:1, parity * 1024:(parity + 1) * 1024])
                    # fp16 rounding for the broadcast matmul (SBUF->SBUF cast
                    # on the otherwise idle Pool engine)
                    nc.gpsimd.tensor_copy(
                        rrow16_sb[b][0:1, parity * 1024:(parity + 1) * 1024],
                        rrow_sb[b][0:1, parity * 1024:(parity + 1) * 1024])
                    rdb = pt(f"rdb{b}{parity}")
                    for j in range(2):  # matmul psum output <= 512 f32 (1 bank)
                        nc.tensor.matmul(
                            rdb[0:64, j * 512:(j + 1) * 512],
                            ones64_16[0:1, :],
                            rrow16_sb[b][0:1, parity * 1024 + j * 512:parity * 1024 + (j + 1) * 512],
                            start=True, stop=True)
                    nc.vector.tensor_tensor(
                        ao_sb[b][parity * 64:parity * 64 + 64, :],
                        aoun_sb[b][parity * 64:parity * 64 + 64, :],
                        rdb[0:64, 0:1024],
                        mybir.AluOpType.mult)

            def emit_outproj(b):
                tP = pt(f"tP{b}")
                for qc in range(2):
                    ps = tP[:, qc * 512:(qc + 1) * 512]
                    for ec in range(4):
                        nc.tensor.matmul(ps, ao_sb[b][:, ec * QC + 128 * qc:ec * QC + 128 * qc + 128],
                                         wsl("o", ec, 0, 512), start=(ec == 0),
                                         stop=(ec == 3 and not with_bias))
                    if with_bias:
                        nc.tensor.matmul(ps, ones16[0:1, 0:128], b_sb["o"][:],
                                         start=False, stop=True)
                # single staging copy + single DMA for both query halves
                nc.scalar.copy(osb[b][:], tP[:])
                nc.sync.dma_start(
                    out[b].rearrange("(c p) e -> p c e", c=2),
                    osb[b][:].rearrange("p (c e) -> p c e", c=2))

            # ---- main body: interleave the two batches ------------------------
            for rep in range(reps):
                emit_qproj(0)
                emit_kproj(0)
                emit_vproj(0, [0, 1])
                emit_scores(0)
                emit_vproj(0, [2])
                # pre-allocate b0's attnV tiles: they recycle b0's own score
                # slots (retired by b0 exps) instead of b1's (retired late)
                tOs0 = (pt("tOe0"), pt("tOo0"))
                emit_qproj(1)
                emit_kproj(1)
                emit_vproj(1, [0, 1])
                emit_scores(1)
                emit_vproj(1, [2])
                emit_attnv(0, tOs0)
                emit_drow(0, tOs0)
                emit_norm(0, tOs0)
                tOs1 = emit_attnv(1)
                emit_drow(1, tOs1)
                emit_outproj(0)
                emit_norm(1, tOs1)
                emit_outproj(1)

    nc.compile()
    return nc


def host_prep(x, padding_mask, Wqkv, bqkv, Wo, bo):
    """Build per-core input maps (numpy only)."""
    x = np.asarray(x, dtype=np.float32)
    pm = np.asarray(padding_mask) != 0
    Wqkv = np.asarray(Wqkv, dtype=np.float32)
    bqkv = np.asarray(bqkv, dtype=np.float32)
    Wo_np = np.asarray(Wo, dtype=np.float32)
    bo_np = np.asarray(bo, dtype=np.float32)

    def pack_w(w):
        # [512, E] -> [128, 4*E]: contraction chunk kc along the free dim
        return np.ascontiguousarray(
            w.reshape(4, 128, EMBED).transpose(1, 0, 2).reshape(128, 4 * EMBED)
        ).astype(np.float16)

    hidx = np.arange(HEADS).repeat(64) * 192 + np.tile(np.arange(64), HEADS)
    Wq = pack_w(Wqkv[:, hidx] / 8.0)
    Wk = pack_w(Wqkv[:, hidx + 64])
    Wv = pack_w(Wqkv[:, hidx + 128])
    Wo16 = pack_w(Wo_np)
    with_bias = bool(bqkv.any() or bo_np.any())
    bq = (bqkv[hidx] / 8.0).astype(np.float16)[None, :]
    bk = bqkv[hidx + 64].astype(np.float16)[None, :]
    bv = bqkv[hidx + 128].astype(np.float16)[None, :]
    bo2 = bo_np.astype(np.float16)[None, :]

    x_pad = np.zeros((B, S + 2 * HALF, IN_DIM), np.float32)
    x_pad[:, HALF:HALF + S] = x
    pm_pad = np.zeros((B, S + 2 * HALF), bool)
    pm_pad[:, HALF:HALF + S] = pm

    in_maps = []
    for c in range(N_CORES):
        q0 = QC * c
        xT_c = np.ascontiguousarray(
            x_pad[:, q0:q0 + KW, :].transpose(0, 2, 1)  # [B, 512, KW]
            .reshape(B, 4, 128, KW).transpose(0, 2, 1, 3)
            .reshape(B, 128, 4 * KW)).astype(np.float16)
        km_c = np.zeros((B, 384), np.float32)
        km_c[:, :KW] = np.where(pm_pad[:, q0:q0 + KW], 0.0, -1e9)
        km_c[:, KW:] = -1e9
        band_c = np.zeros((B, 128, BAND_W), np.float16)
        for ci, (kw, rq0, rw, boff) in enumerate(CHUNKS):
            kk = np.arange(128)[:, None]
            jj = np.arange(rw)[None, :]
            krel = 128 * ci + kk
            qq = rq0 + jj
            geo = (krel - qq >= 0) & (krel - qq <= 64) & (kk < kw)
            qpad = pm[:, q0 + rq0:q0 + rq0 + rw]  # [B, rw]
            band_c[:, :, boff:boff + rw] = (geo[None] * qpad[:, None, :]).astype(np.float16)
        m = {
            "xT": xT_c, "Wq": Wq, "Wk": Wk, "Wv": Wv, "Wo": Wo16,
            "km": km_c, "band": band_c,
        }
        if with_bias:
            m.update({"bq": bq, "bk": bk, "bv": bv, "bo": bo2})
        in_maps.append(m)
    return in_maps, with_bias


_NC_CACHE = {}


def kernel(x, padding_mask, Wqkv, bqkv, Wo, bo):
    in_maps, with_bias = host_prep(x, padding_mask, Wqkv, bqkv, Wo, bo)
    key = ("nc", with_bias)
    if key not in _NC_CACHE:
        _NC_CACHE[key] = build_nc(reps=1, with_bias=with_bias)
    nc = _NC_CACHE[key]
    res = run_bass_kernel_spmd(nc, in_maps, core_ids=list(range(N_CORES)), trace=False)
    full = np.empty((B, S, EMBED), np.float32)
    for c in range(N_CORES):
        full[:, QC * c:QC * (c + 1), :] = res.results[c]["out"]
    return full


# revision 5
# speedup vs baseline: 3.2477x; 1.0894x over previous
"""Sliding-window MHA (B=2, S=2048, E=512, H=8, Dh=64, W=64+1) on 8 trn2
NeuronCores, sequence-parallel (256 queries/core + 32-halo), weights replicated.

v2: fp16 matmul path (PE: 1 cyc/row at any free size), rect-tight score and
attnV matmuls (6-split banded accumulation), softmax normalization via
sbuf-row reciprocal_approx_fast + fp16 rank-1 PE broadcast + single-PSUM
multiply, PSUM->SBUF moves spread across ACT/DVE (GPSIMD cannot touch PSUM),
batches interleaved in emission order so PE stays busy during ACT/DVE phases.
All matmul PSUM outputs are partition-0, single-bank, non-crossing (HW req).

Layouts per core:
  qT  [128, 4*256] fp16: partition=(h%2)*64+d, col block m=h//2 (E-major)
  kT  [128, 4*320] fp16: same, KW=320 keys
  V   [sw, 8*65]   fp16 per 128-key chunk: col h*65+d, col h*65+64 = ones
  attnT[ci] [128, 8*rw] fp16: col g*rw+q-rect, g=(h%2)*4+h//2
  scores psum per chunk: even heads g0..3 / odd g4..7 in bank-disjoint ranges
  tO [65, 4*256] f32 psum (e/o): row 64 = softmax denominator
  ao [128, 4*256] fp16: E-major normalized attention output
  out-proj: S-major psum [128 q, 512] -> sbuf f32 -> DMA
"""
import numpy as np

import concourse.bass as bass
import concourse.tile as tile
from concourse import bacc, mybir
from concourse.bass_utils import run_bass_kernel_spmd

F32 = mybir.dt.float32
F32R = mybir.dt.float32r
F16 = mybir.dt.float16

B, S, IN_DIM, EMBED, HEADS, WINDOW = 2, 2048, 512, 512, 8, 64
HALF = WINDOW // 2          # 32
N_CORES = 8
QC = S // N_CORES           # 256 queries per core
KW = QC + 2 * HALF          # 320 key rows per core window
EPS = 2e-5

# chunks along the key axis: (kw, rect_q0, rect_w, band_col_off)
CHUNKS = [(128, 0, 128, 0), (128, 64, 192, 128), (64, 192, 64, 320)]
BAND_W = 384
# attnV accumulation regions: (q0, ci, rect_off, start, stop); width 64 each
AV_REGIONS = [
    (0, 0, 0, True, True),
    (64, 0, 64, True, False), (64, 1, 0, False, True),
    (128, 1, 64, True, True),
    (192, 1, 128, True, False), (192, 2, 0, False, True),
]
# av_mode="wide": attnT[1] zero-padded to [128, 8*256]; 3 matmuls per head
# (c1 full-width first with start, c0/c2 tight adds)
AV_WIDE = [
    (1, 0, 256, True, False),   # (ci, q0, width, start, stop); rect_off = q0-rq0
    (0, 0, 128, False, False),
    (2, 192, 64, False, True),
]


def build_nc(reps: int = 1, with_bias: bool = False, av_mode: str = "split"):
    nc = bacc.Bacc("TRN2", target_bir_lowering=False, debug=False, num_devices=N_CORES)

    # packed layouts: contraction-chunk kc along the free dim (one DMA each)
    xT = nc.dram_tensor("xT", [B, 128, 4 * KW], F16, kind="ExternalInput").ap()
    Wq = nc.dram_tensor("Wq", [128, 4 * EMBED], F16, kind="ExternalInput").ap()
    Wk = nc.dram_tensor("Wk", [128, 4 * EMBED], F16, kind="ExternalInput").ap()
    Wv = nc.dram_tensor("Wv", [128, 4 * EMBED], F16, kind="ExternalInput").ap()
    Wo = nc.dram_tensor("Wo", [128, 4 * EMBED], F16, kind="ExternalInput").ap()
    km = nc.dram_tensor("km", [B, 384], F32, kind="ExternalInput").ap()
    band = nc.dram_tensor("band", [B, 128, BAND_W], F16, kind="ExternalInput").ap()
    out = nc.dram_tensor("out", [B, QC, EMBED], F32, kind="ExternalOutput").ap()
    if with_bias:
        bq = nc.dram_tensor("bq", [1, EMBED], F16, kind="ExternalInput").ap()
        bk = nc.dram_tensor("bk", [1, EMBED], F16, kind="ExternalInput").ap()
        bv = nc.dram_tensor("bv", [1, EMBED], F16, kind="ExternalInput").ap()
        bo = nc.dram_tensor("bo", [1, EMBED], F16, kind="ExternalInput").ap()

    with tile.TileContext(nc) as tc:
        with tc.tile_pool(name="const", bufs=1) as cpool, \
             tc.tile_pool(name="work", bufs=1) as wpool, \
             tc.tile_pool(name="psum", bufs=4, space="PSUM") as ppool:

            def pt(name):
                return ppool.tile([128, 1024], F32, tag="big", name=name)

            # ---- constant loads (consumption order, packed) -------------------
            W_sb, xT_sb, km_sb, band_sb = {}, {}, {}, {}

            def load_w(name, src):
                t = cpool.tile([128, 4 * EMBED], F16, tag=f"W{name}", name=f"W{name}")
                nc.sync.dma_start(t[:], src[:])
                W_sb[name] = t

            def wsl(name, kc, c0, w):
                return W_sb[name][:, kc * EMBED + c0:kc * EMBED + c0 + w]

            def load_x(b):
                # second HWDGE queue (Activation) runs in parallel with SP
                t = cpool.tile([128, 4 * KW], F16, tag=f"xT{b}", name=f"xT{b}")
                nc.scalar.dma_start(t[:], xT[b])
                xT_sb[b] = t

            def xsl(b, kc, c0, w):
                return xT_sb[b][:, kc * KW + c0:kc * KW + c0 + w]

            def load_masks(b):
                t = cpool.tile([128, 3], F32, tag=f"km{b}", name=f"km{b}")
                nc.scalar.dma_start(t[:], km[b].rearrange("(c p) -> p c", p=128))
                km_sb[b] = t
                t = cpool.tile([128, BAND_W], F16, tag=f"band{b}", name=f"band{b}")
                nc.scalar.dma_start(t[:], band[b])
                band_sb[b] = t

            load_w("q", Wq)
            load_x(0)
            load_w("k", Wk)
            load_w("v", Wv)
            load_masks(0)
            load_w("o", Wo)
            load_x(1)
            load_masks(1)
            if with_bias:
                b_sb = {}
                for name, src in (("q", bq), ("k", bk), ("v", bv), ("o", bo)):
                    t = cpool.tile([1, EMBED], F16, tag=f"b{name}", name=f"b{name}")
                    nc.sync.dma_start(t[:], src[:])
                    b_sb[name] = t

            # scratch constants
            onescr = wpool.tile([128, 64], F32, tag="onescr")
            nc.vector.memset(onescr[:], 1.0)
            ones64_16 = cpool.tile([1, 64], F16, tag="ones64h")
            nc.vector.tensor_copy(ones64_16[:], onescr[0:1, :])
            if with_bias:
                ones16 = cpool.tile([1, QC], F16, tag="ones16")
                nc.vector.memset(ones16[:], 1.0)

            # V tiles with ones-columns at h*65+64 (set once)
            V_sb = {}
            for b in range(B):
                for sc in range(3):
                    t = cpool.tile([128, 8 * 65], F16, tag=f"V{b}{sc}", name=f"V{b}{sc}")
                    nc.vector.tensor_copy(
                        t[:].rearrange("p (h c) -> p h c", h=8)[:, :, 64:65],
                        onescr[:, 0:1].rearrange("p (o w) -> p o w", o=1).broadcast_to([128, 8, 1]))
                    V_sb[b, sc] = t

            # attnT per chunk fp16 (rects fully rewritten each rep). In "wide"
            # av_mode chunk 1 is zero-padded to full 256-wide head blocks so a
            # single full-width matmul with start=True opens each head's
            # accumulation; the pad region is never written after setup.
            zscr = wpool.tile([128, 2048], F32, tag="zscr")
            nc.vector.memset(zscr[:], 0.0)
            attnT = {}
            AT_W = {}   # head-block width per chunk
            AT_OFF = {}  # col offset of rect within block
            for ci, (kw, rq0, rw, boff) in enumerate(CHUNKS):
                for b in range(B):
                    if av_mode == "wide" and ci == 1:
                        AT_W[ci], AT_OFF[ci] = 256, 64
                        t = cpool.tile([128, 8 * 256], F16, tag=f"attnT{b}{ci}", name=f"attnT{b}{ci}")
                        nc.vector.tensor_copy(t[:], zscr[:])
                    else:
                        AT_W[ci], AT_OFF[ci] = rw, 0
                        t = cpool.tile([128, 8 * rw], F16, tag=f"attnT{b}{ci}", name=f"attnT{b}{ci}")
                    attnT[b, ci] = t

            qT_sb = [cpool.tile([128, 4 * QC], F16, tag=f"qT{b}", name=f"qT{b}") for b in range(B)]
            kT_sb = [cpool.tile([128, 4 * KW], F16, tag=f"kT{b}", name=f"kT{b}") for b in range(B)]
            # unnormalized attention output staging (psum->sbuf: tensor ops
            # may read at most one PSUM operand, and this frees tO slots early)
            aoun_sb = [cpool.tile([128, 4 * QC], F32, tag=f"aoun{b}", name=f"aoun{b}")
                       for b in range(B)]
            ao_sb = [cpool.tile([128, 4 * QC], F16, tag=f"ao{b}", name=f"ao{b}") for b in range(B)]
            drow_sb = [cpool.tile([1, 2048], F32, tag=f"drow{b}", name=f"drow{b}") for b in range(B)]
            rrow_sb = [cpool.tile([1, 2048], F32, tag=f"rrow{b}", name=f"rrow{b}") for b in range(B)]
            rrow16_sb = [cpool.tile([1, 2048], F16, tag=f"rrow16{b}", name=f"rrow16{b}") for b in range(B)]
            osb = [cpool.tile([128, 2 * EMBED], F32, tag=f"osb{b}", name=f"osb{b}")
                   for b in range(B)]

            # score psum column offsets per chunk for even/odd head groups
            SCOL = {0: (0, 512), 1: (0, 0), 2: (0, 512)}  # c1 uses two tiles

            # ---------- emission helpers ----------
            def emit_qproj(b):
                tQ = pt(f"tQ{b}")
                for m in range(4):
                    ps = tQ[:, m * 256:m * 256 + QC]
                    for kc in range(4):
                        nc.tensor.matmul(ps, wsl("q", kc, 128 * m, 128),
                                         xsl(b, kc, HALF, QC),
                                         start=(kc == 0), stop=(kc == 3 and not with_bias))
                    if with_bias:
                        nc.tensor.matmul(ps, b_sb["q"][0:1, 128 * m:128 * m + 128],
                                         ones16[0:1, :], start=False, stop=True)
                nc.scalar.copy(qT_sb[b][:], tQ[:])

            def emit_kproj(b):
                for half in range(2):
                    tK = pt(f"tK{b}{half}")
                    for mi in range(2):
                        m = half * 2 + mi
                        ps = tK[:, mi * 512:mi * 512 + KW]  # bank-aligned
                        for kc in range(4):
                            nc.tensor.matmul(ps, wsl("k", kc, 128 * m, 128),
                                             xsl(b, kc, 0, KW),
                                             start=(kc == 0), stop=(kc == 3 and not with_bias))
                        if with_bias:
                            nc.tensor.matmul(ps, b_sb["k"][0:1, 128 * m:128 * m + 128],
                                             ones16[0:1, 0:KW], start=False, stop=True)
                    nc.vector.tensor_copy(
                        kT_sb[b][:, half * 2 * KW:(half + 1) * 2 * KW].rearrange(
                            "p (c k) -> p c k", c=2),
                        tK[:].rearrange("p (c k) -> p c k", c=2)[:, :, 0:KW])

            def emit_vproj(b, scs):
                # V projection for the listed 128-key chunks (one psum slot)
                tV = pt(f"tV{b}{scs[0]}")
                for i, sc in enumerate(scs):
                    sw = min(128, KW - 128 * sc)
                    ps = tV[0:sw, i * 512:i * 512 + 512]
                    for kc in range(4):
                        nc.tensor.matmul(ps, xsl(b, kc, 128 * sc, sw),
                                         wsl("v", kc, 0, 512), start=(kc == 0),
                                         stop=(kc == 3 and not with_bias))
                    if with_bias:
                        nc.tensor.matmul(ps, ones16[0:1, 0:sw], b_sb["v"][:],
                                         start=False, stop=True)
                    nc.vector.tensor_copy(
                        V_sb[b, sc][0:sw].rearrange("p (h c) -> p h c", h=8)[:, :, 0:64],
                        ps.rearrange("p (h d) -> p h d", h=8))

            def emit_exp_mask(b, ci, tS, soff, kw, rw, boff, nh, gofs, stride=None):
                # exp(+key-mask bias) then band multiply for nh head blocks;
                # soff = offset of the first block WITHIN a stride-sized slot
                st = stride if stride is not None else rw
                in3 = tS[0:kw, 0:nh * st].rearrange(
                    "p (h q) -> p h q", h=nh)[:, :, soff:soff + rw] if st != rw else \
                    tS[0:kw, soff:soff + nh * rw].rearrange("p (h q) -> p h q", h=nh)
                aw, aoff = AT_W[ci], AT_OFF[ci]
                out3 = attnT[b, ci][0:kw, gofs * aw:(gofs + nh) * aw].rearrange(
                    "p (h q) -> p h q", h=nh)[:, :, aoff:aoff + rw]
                nc.scalar.activation(out3, in3, mybir.ActivationFunctionType.Exp,
                                     bias=km_sb[b][0:kw, ci:ci + 1])
                b3 = band_sb[b][0:kw, boff:boff + rw].rearrange(
                    "p (o w) -> p o w", o=1).broadcast_to([kw, nh, rw])
                nc.vector.tensor_tensor(out3, out3, b3, mybir.AluOpType.mult)

            def emit_score_mms(b, ci, tiles, offs, stride=None):
                kw, rq0, rw, _ = CHUNKS[ci]
                st = stride if stride is not None else rw
                for h in range(8):
                    bp = (h % 2) * 64
                    tS, off = tiles[h % 2], offs[h % 2]
                    nc.tensor.matmul(
                        tS[0:kw, off + (h // 2) * st: off + (h // 2) * st + rw],
                        kT_sb[b][bp:bp + 64, (h // 2) * KW + 128 * ci:(h // 2) * KW + 128 * ci + kw],
                        qT_sb[b][bp:bp + 64, (h // 2) * QC + rq0:(h // 2) * QC + rq0 + rw],
                        start=True, stop=True)

            def emit_scores(b):
                # c0: one tile, even [0:512] / odd [512:1024] (bank-disjoint),
                # merged exp+mask across all 8 blocks.
                tS0 = pt(f"tS{b}0")
                emit_score_mms(b, 0, (tS0, tS0), (0, 512))
                emit_exp_mask(b, 0, tS0, 0, 128, 128, 0, 8, 0)
                # c1: two tiles, blocks at 256-stride (width 192, no bank
                # crossing); c2 rides in the 64-wide gaps at +192.
                tS1e, tS1o = pt(f"tS{b}1e"), pt(f"tS{b}1o")
                emit_score_mms(b, 1, (tS1e, tS1o), (0, 0), stride=256)
                emit_exp_mask(b, 1, tS1e, 0, 128, 192, 128, 4, 0, stride=256)
                emit_exp_mask(b, 1, tS1o, 0, 128, 192, 128, 4, 4, stride=256)
                emit_score_mms(b, 2, (tS1e, tS1o), (192, 192), stride=256)
                emit_exp_mask(b, 2, tS1e, 192, 64, 64, 320, 4, 0, stride=256)
                emit_exp_mask(b, 2, tS1o, 192, 64, 64, 320, 4, 4, stride=256)

            def emit_attnv(b, tOs=None):
                if tOs is None:
                    tOs = (pt(f"tOe{b}"), pt(f"tOo{b}"))
                # evens first: the odd tile's slot frees later (WAR on b1 exps)
                # and drow-e fires as soon as the even half is done
                for h in (0, 2, 4, 6, 1, 3, 5, 7):
                    g = (h % 2) * 4 + h // 2
                    tO = tOs[h % 2]
                    if av_mode == "split":
                        for (q0, ci, roff, s, e) in AV_REGIONS:
                            kw_c = CHUNKS[ci][0]
                            nc.tensor.matmul(
                                tO[0:65, (h // 2) * QC + q0:(h // 2) * QC + q0 + 64],
                                V_sb[b, ci][0:kw_c, h * 65:(h + 1) * 65],
                                attnT[b, ci][0:kw_c, g * AT_W[ci] + roff:g * AT_W[ci] + roff + 64],
                                start=s, stop=e)
                    else:
                        for (ci, q0, w, s, e) in AV_WIDE:
                            kw_c = CHUNKS[ci][0]
                            bc = AT_OFF[ci] + q0 - CHUNKS[ci][1]
                            nc.tensor.matmul(
                                tO[0:65, (h // 2) * QC + q0:(h // 2) * QC + q0 + w],
                                V_sb[b, ci][0:kw_c, h * 65:(h + 1) * 65],
                                attnT[b, ci][0:kw_c, g * AT_W[ci] + bc:g * AT_W[ci] + bc + w],
                                start=s, stop=e, skip_group_check=True)
                return tOs

            def emit_drow(b, tOs):
                # denominator rows -> sbuf f32r (+eps) on ACT; unnormalized
                # attention output -> sbuf (ACT even / DVE odd), freeing the
                # tO psum slots early
                nc.scalar.activation(drow_sb[b][0:1, 0:1024], tOs[0][64:65, :],
                                     mybir.ActivationFunctionType.Copy, bias=EPS)
                nc.scalar.copy(aoun_sb[b][0:64, :], tOs[0][0:64, :])
                nc.scalar.activation(drow_sb[b][0:1, 1024:2048], tOs[1][64:65, :],
                                     mybir.ActivationFunctionType.Copy, bias=EPS)
                nc.scalar.copy(aoun_sb[b][64:128, :], tOs[1][0:64, :])

            def emit_norm(b, tOs):
                # rrow = 1/denom on the SBUF row (proven op placement), then
                # PE-broadcast reciprocals to psum, then ao = aoun * rdb
                # (one PSUM operand per tensor_tensor)
                for parity in range(2):
                    nc.vector.reciprocal_approx_fast(
                        rrow_sb[b][0:1, parity * 1024:(parity + 1) * 1024],
                        drow_sb[b][0:1, parity * 1024:(parity + 1) * 1024])
                    # fp16 rounding for the broadcast matmul
                    nc.vector.tensor_copy(
                        rrow16_sb[b][0:1, parity * 1024:(parity + 1) * 1024],
                        rrow_sb[b][0:1, parity * 1024:(parity + 1) * 1024])
                    rdb = pt(f"rdb{b}{parity}")
                    for j in range(2):  # matmul psum output <= 512 f32 (1 bank)
                        nc.tensor.matmul(
                            rdb[0:64, j * 512:(j + 1) * 512],
                            ones64_16[0:1, :],
                            rrow16_sb[b][0:1, parity * 1024 + j * 512:parity * 1024 + (j + 1) * 512],
                            start=True, stop=True)
                    nc.vector.tensor_tensor(
                        ao_sb[b][parity * 64:parity * 64 + 64, :],
                        aoun_sb[b][parity * 64:parity * 64 + 64, :],
                        rdb[0:64, 0:1024],
                        mybir.AluOpType.mult)

            def emit_outproj(b):
                tP = pt(f"tP{b}")
                for qc in range(2):
                    ps = tP[:, qc * 512:(qc + 1) * 512]
                    for ec in range(4):
                        nc.tensor.matmul(ps, ao_sb[b][:, ec * QC + 128 * qc:ec * QC + 128 * qc + 128],
                                         wsl("o", ec, 0, 512), start=(ec == 0),
                                         stop=(ec == 3 and not with_bias))
                    if with_bias:
                        nc.tensor.matmul(ps, ones16[0:1, 0:128], b_sb["o"][:],
                                         start=False, stop=True)
                # single staging copy + single DMA for both query halves
                nc.scalar.copy(osb[b][:], tP[:])
                nc.sync.dma_start(
                    out[b].rearrange("(c p) e -> p c e", c=2),
                    osb[b][:].rearrange("p (c e) -> p c e", c=2))

            # ---- main body: interleave the two batches ------------------------
            for rep in range(reps):
                emit_qproj(0)
                emit_kproj(0)
                emit_vproj(0, [0, 1])
                emit_scores(0)
                emit_vproj(0, [2])
                # pre-allocate b0's attnV tiles: they recycle b0's own score
                # slots (retired by b0 exps) instead of b1's (retired late)
                tOs0 = (pt("tOe0"), pt("tOo0"))
                emit_qproj(1)
                emit_kproj(1)
                emit_vproj(1, [0, 1])
                emit_scores(1)
                emit_vproj(1, [2])
                emit_attnv(0, tOs0)
                emit_drow(0, tOs0)
                emit_norm(0, tOs0)
                tOs1 = emit_attnv(1)
                emit_drow(1, tOs1)
                emit_outproj(0)
                emit_norm(1, tOs1)
                emit_outproj(1)

    nc.compile()
    return nc


def host_prep(x, padding_mask, Wqkv, bqkv, Wo, bo):
    """Build per-core input maps (numpy only)."""
    x = np.asarray(x, dtype=np.float32)
    pm = np.asarray(padding_mask) != 0
    Wqkv = np.asarray(Wqkv, dtype=np.float32)
    bqkv = np.asarray(bqkv, dtype=np.float32)
    Wo_np = np.asarray(Wo, dtype=np.float32)
    bo_np = np.asarray(bo, dtype=np.float32)

    def pack_w(w):
        # [512, E] -> [128, 4*E]: contraction chunk kc along the free dim
        return np.ascontiguousarray(
            w.reshape(4, 128, EMBED).transpose(1, 0, 2).reshape(128, 4 * EMBED)
        ).astype(np.float16)

    hidx = np.arange(HEADS).repeat(64) * 192 + np.tile(np.arange(64), HEADS)
    Wq = pack_w(Wqkv[:, hidx] / 8.0)
    Wk = pack_w(Wqkv[:, hidx + 64])
    Wv = pack_w(Wqkv[:, hidx + 128])
    Wo16 = pack_w(Wo_np)
    with_bias = bool(bqkv.any() or bo_np.any())
    bq = (bqkv[hidx] / 8.0).astype(np.float16)[None, :]
    bk = bqkv[hidx + 64].astype(np.float16)[None, :]
    bv = bqkv[hidx + 128].astype(np.float16)[None, :]
    bo2 = bo_np.astype(np.float16)[None, :]

    x_pad = np.zeros((B, S + 2 * HALF, IN_DIM), np.float32)
    x_pad[:, HALF:HALF + S] = x
    pm_pad = np.zeros((B, S + 2 * HALF), bool)
    pm_pad[:, HALF:HALF + S] = pm

    in_maps = []
    for c in range(N_CORES):
        q0 = QC * c
        xT_c = np.ascontiguousarray(
            x_pad[:, q0:q0 + KW, :].transpose(0, 2, 1)  # [B, 512, KW]
            .reshape(B, 4, 128, KW).transpose(0, 2, 1, 3)
            .reshape(B, 128, 4 * KW)).astype(np.float16)
        km_c = np.zeros((B, 384), np.float32)
        km_c[:, :KW] = np.where(pm_pad[:, q0:q0 + KW], 0.0, -1e9)
        km_c[:, KW:] = -1e9
        band_c = np.zeros((B, 128, BAND_W), np.float16)
        for ci, (kw, rq0, rw, boff) in enumerate(CHUNKS):
            kk = np.arange(128)[:, None]
            jj = np.arange(rw)[None, :]
            krel = 128 * ci + kk
            qq = rq0 + jj
            geo = (krel - qq >= 0) & (krel - qq <= 64) & (kk < kw)
            qpad = pm[:, q0 + rq0:q0 + rq0 + rw]  # [B, rw]
            band_c[:, :, boff:boff + rw] = (geo[None] * qpad[:, None, :]).astype(np.float16)
        m = {
            "xT": xT_c, "Wq": Wq, "Wk": Wk, "Wv": Wv, "Wo": Wo16,
            "km": km_c, "band": band_c,
        }
        if with_bias:
            m.update({"bq": bq, "bk": bk, "bv": bv, "bo": bo2})
        in_maps.append(m)
    return in_maps, with_bias


_NC_CACHE = {}


def kernel(x, padding_mask, Wqkv, bqkv, Wo, bo):
    in_maps, with_bias = host_prep(x, padding_mask, Wqkv, bqkv, Wo, bo)
    key = ("nc", with_bias)
    if key not in _NC_CACHE:
        _NC_CACHE[key] = build_nc(reps=1, with_bias=with_bias)
    nc = _NC_CACHE[key]
    res = run_bass_kernel_spmd(nc, in_maps, core_ids=list(range(N_CORES)), trace=False)
    full = np.empty((B, S, EMBED), np.float32)
    for c in range(N_CORES):
        full[:, QC * c:QC * (c + 1), :] = res.results[c]["out"]
    return full
